# revision 13
# baseline (speedup 1.0000x reference)
"""Trainium2 Bass kernel for nn_Net_71330816852788 (3-layer GCSConv GNN + pool).

Self-contained: takes FULL inputs, shards across 8 NeuronCores internally,
returns the FULL [4096, 1] output.

Strategy (data-parallel over destination nodes):
  - nodes sharded 32768/core; each core aggregates messages for its own nodes
  - per-core edge lists organized into a degree-sorted "level" structure so
    aggregation is a chain of identity-matmuls accumulating in PSUM
  - gather of source rows via indirect DMA (128 rows/call) from a full
    replicated node-feature table; tables shared between layers via AllGather
  - graph pooling on-device via per-tile selection matmuls, cross-core
    boundary graphs resolved through a small partial-sum AllGather
"""
import sys
import types

import numpy as np

import concourse.bass as bass
import concourse.bacc as bacc
import concourse.mybir as mybir
from concourse import tile
from concourse.bass_utils import run_bass_kernel_spmd
from concourse.masks import make_identity

N = 262144
E = 4194304
G = 4096
CH = 32
NC = 8
NS = N // NC          # 32768 nodes per core
P = 128
TILES = NS // P       # 256
CL = 16               # levels per gather chunk
GW = 768              # pooling partial-window width (graphs)
GSH = G // NC         # 512 graphs per core

TRACE = False
LAST_EXEC_NS = None

f32 = mybir.dt.float32
i32 = mybir.dt.int32


def _install_ntff_hook():
    """Optional: shim the missing antenv.axon_hooks so trace=True works."""
    try:
        if "antenv.axon_hooks" in sys.modules:
            return True
        mod = types.ModuleType("antenv.axon_hooks")
        mod._hook = None
        mod.set_axon_ntff_profile_hook = lambda h: setattr(mod, "_hook", h)
        mod.get_axon_ntff_profile_hook = lambda: mod._hook
        sys.modules["antenv.axon_hooks"] = mod
        import antenv

        antenv.axon_hooks = mod
        from trn_agent_boot.trn_boot import _ntff_profile_via_ctypes

        mod.set_axon_ntff_profile_hook(
            _ntff_profile_via_ctypes("/opt/axon/libaxon_pjrt.so")
        )
        return True
    except Exception:
        return False


def _preprocess(x, edge_src, edge_dst, edge_w, seg):
    """All index-side preprocessing (numpy). Returns per-core inputs + meta."""
    x = np.asarray(x, np.float32).reshape(N)
    es = np.asarray(edge_src, np.int64)
    ed = np.asarray(edge_dst, np.int64)
    ew = np.asarray(edge_w, np.float32)
    seg = np.asarray(seg, np.int64)

    deg_all = np.bincount(ed, minlength=N)

    # per-core degree-sorted node permutation
    perms = []
    pos_all = np.empty(N, np.int64)
    for c in range(NC):
        d = deg_all[c * NS:(c + 1) * NS]
        perm = np.argsort(-d, kind="stable")
        perms.append(perm)
        pos_all[c * NS + perm] = np.arange(NS)
    rowmap = (np.arange(N) // NS) * NS + pos_all  # original node id -> table row

    # per-tile level counts, equalized across cores
    Lt = np.zeros(TILES, np.int64)
    deg_sorted = []
    for c in range(NC):
        ds = deg_all[c * NS:(c + 1) * NS][perms[c]]
        deg_sorted.append(ds)
        Lt = np.maximum(Lt, ds[::P][:TILES])  # max deg in tile = first of each 128
    levelbase = np.concatenate([[0], np.cumsum(Lt)])
    Ltot = int(levelbase[-1])
    NCHK = (Ltot + CL - 1) // CL
    Lpad = NCHK * CL

    src_row_all = rowmap[es]
    dst_core = ed // NS
    dst_local = ed - dst_core * NS

    in_maps = []
    for c in range(NC):
        sel = dst_core == c
        s_r = src_row_all[sel]
        d_l = dst_local[sel]
        w_c = ew[sel]
        slot = pos_all[c * NS + d_l]          # permuted slot of dst node
        order = np.argsort(slot, kind="stable")
        s_r = s_r[order]
        w_c = w_c[order]
        slot_s = slot[order]
        # rank of each edge within its node's run
        ds = deg_sorted[c]
        starts = np.concatenate([[0], np.cumsum(ds)])[:-1]
        n_e = len(s_r)
        rank = np.arange(n_e) - np.repeat(starts, ds)
        t_of = slot_s // P
        col = slot_s % P
        lvl = levelbase[t_of] + rank          # global level row
        idxarr = np.zeros((Lpad, P), np.int32)
        warr = np.zeros((Lpad, P), np.float32)
        idxarr[lvl, col] = s_r
        warr[lvl, col] = w_c
        idx_in = np.ascontiguousarray(
            idxarr.reshape(NCHK, CL, P).transpose(0, 2, 1))
        w_in = np.ascontiguousarray(
            warr.reshape(NCHK, CL, P).transpose(0, 2, 1))

        # transposed padded x0 for this core (only channel 0 nonzero)
        xT0 = np.zeros((CH, NS), np.float32)
        xT0[0, :] = x[c * NS + perms[c]]

        # pooling one-hot columns: graph column of node at (tile, partition)
        gbase = GSH * c - 128
        gcol = seg[c * NS + perms[c]].astype(np.int64) - gbase
        assert gcol.min() >= 0 and gcol.max() < GW, (
            f"pool window overflow core {c}: {gcol.min()} {gcol.max()}")
        gcol_in = gcol.reshape(TILES, P, 1).astype(np.float32)

        in_maps.append({
            "idx": idx_in, "w": w_in, "xT0": xT0, "gcol": gcol_in,
        })

    # permuted x0 table (layer-0 gather source), same for all cores
    x0p = np.zeros((N, CH), np.float32)
    x0p[rowmap, 0] = x
    cnt = np.bincount(seg, minlength=G).astype(np.float32)
    invcnt = (1.0 / np.maximum(cnt, 1.0)).reshape(G, 1).astype(np.float32)

    # combine contributors per 128-graph output chunk
    contrib = []
    for k in range(G // P):
        lst = []
        for c2 in range(NC):
            off = P * k - (GSH * c2 - 128)
            if 0 <= off and off + P <= GW:
                lst.append((c2, off))
        assert lst
        contrib.append(lst)

    meta = dict(Lt=Lt, levelbase=levelbase, Ltot=Ltot, NCHK=NCHK,
                contrib=contrib)
    shared = dict(x0p=x0p, invcnt=invcnt)
    return in_maps, shared, meta


def _build(meta, weights):
    Lt = meta["Lt"]
    levelbase = meta["levelbase"]
    Ltot = meta["Ltot"]
    NCHK = meta["NCHK"]
    contrib = meta["contrib"]

    nc = bacc.Bacc(None, target_bir_lowering=False, num_devices=NC)

    x0p_ext = nc.declare_dram_parameter("x0p", [N, CH], f32, isOutput=False)
    idx_ext = nc.declare_dram_parameter("idx", [NCHK, P, CL], i32, isOutput=False)
    w_ext = nc.declare_dram_parameter("w", [NCHK, P, CL], f32, isOutput=False)
    xT0_ext = nc.declare_dram_parameter("xT0", [CH, NS], f32, isOutput=False)
    gcol_ext = nc.declare_dram_parameter("gcol", [TILES, P, 1], f32, isOutput=False)
    invcnt_ext = nc.declare_dram_parameter("invcnt", [G, 1], f32, isOutput=False)
    wmat_ext = nc.declare_dram_parameter("wmat", [CH, 6, CH], f32, isOutput=False)
    bvec_ext = nc.declare_dram_parameter("bvec", [CH, 3], f32, isOutput=False)
    wd_ext = nc.declare_dram_parameter("wd", [CH, 1], f32, isOutput=False)
    bd_ext = nc.declare_dram_parameter("bd", [P, 1], f32, isOutput=False)
    out_ext = nc.declare_dram_parameter("out", [G, 1], f32, isOutput=True)

    table0 = nc.dram_tensor("table0", [N, CH], f32)
    tables = [table0,
              nc.dram_tensor("table1", [N, CH], f32),
              nc.dram_tensor("table2", [N, CH], f32)]
    ag_in = [nc.dram_tensor(f"ag_in{l}", [NS, CH], f32) for l in range(2)]
    ag_out = [nc.dram_tensor(f"ag_out{l}", [N, CH], f32, addr_space="Shared")
              for l in range(2)]
    xt_dram = [nc.dram_tensor("xt_a", [CH, NS], f32),
               nc.dram_tensor("xt_b", [CH, NS], f32)]
    part_in = nc.dram_tensor("part_in", [GW, CH], f32)
    part_out = nc.dram_tensor("part_out", [NC * GW, CH], f32, addr_space="Shared")
    part_int = nc.dram_tensor("part_int", [NC, GW, CH], f32)

    with tile.TileContext(nc) as tc:
        with (
            tc.tile_pool(name="const", bufs=1) as constp,
            tc.tile_pool(name="sb", bufs=3) as sb,
            tc.tile_pool(name="ep", bufs=3) as ep,
            tc.tile_pool(name="ps", bufs=4, space="PSUM") as ps,
            tc.tile_pool(name="pool_ps", bufs=1, space="PSUM") as pool_ps,
        ):
            ident = constp.tile([P, P], f32)
            make_identity(nc, ident[:])
            zero32 = constp.tile([P, CH], f32)
            nc.vector.memset(zero32[:], 0.0)
            iota_i = constp.tile([P, GW], i32)
            nc.gpsimd.iota(iota_i[:], pattern=[[1, GW]], base=0,
                           channel_multiplier=0)
            iota_f = constp.tile([P, GW], f32)
            nc.vector.tensor_copy(iota_f[:], iota_i[:])

            wmats = constp.tile([CH, 6, CH], f32)
            nc.sync.dma_start(out=wmats[:], in_=wmat_ext[:])
            bvecs = constp.tile([CH, 3], f32)
            nc.sync.dma_start(out=bvecs[:], in_=bvec_ext[:])
            wd_t = constp.tile([CH, 1], f32)
            nc.sync.dma_start(out=wd_t[:], in_=wd_ext[:])
            bd_t = constp.tile([P, 1], f32)
            nc.sync.dma_start(out=bd_t[:], in_=bd_ext[:])

            nc.sync.dma_start(out=table0[:], in_=x0p_ext[:])
            nc.sync.dma_start(out=xt_dram[0][:], in_=xT0_ext[:])

            poolA = pool_ps.tile([CH, 512], f32, space="PSUM")
            poolB = pool_ps.tile([CH, GW - 512], f32, space="PSUM")

            for layer in range(3):
                table_cur = tables[layer]
                msg_t = None
                psum_y = None
                # aggregation over levels
                for t in range(TILES):
                    lt = int(Lt[t])
                    psum_y = ps.tile([P, CH], f32, tag="pp", space="PSUM")
                    if lt == 0:
                        nc.tensor.matmul(psum_y[:], lhsT=ident[:],
                                         rhs=zero32[:], start=True, stop=True)
                    for d in range(lt):
                        l = int(levelbase[t]) + d
                        ck, cc = l // CL, l % CL
                        if cc == 0:
                            idx_t = sb.tile([P, CL], i32, tag="idx")
                            nc.sync.dma_start(out=idx_t[:], in_=idx_ext[ck])
                            w_t = sb.tile([P, CL], f32, tag="w")
                            nc.sync.dma_start(out=w_t[:], in_=w_ext[ck])
                            msg_t = sb.tile([P, CL, CH], f32, tag="msg")
                            for dd in range(CL):
                                if ck * CL + dd >= Ltot:
                                    break
                                nc.gpsimd.indirect_dma_start(
                                    out=msg_t[:, dd, :], out_offset=None,
                                    in_=table_cur[:],
                                    in_offset=bass.IndirectOffsetOnAxis(
                                        ap=idx_t[:, dd:dd + 1], axis=0))
                            nc.vector.tensor_tensor(
                                out=msg_t[:], in0=msg_t[:],
                                in1=w_t[:].to_broadcast([P, CL, CH]),
                                op=mybir.AluOpType.mult)
                        nc.tensor.matmul(psum_y[:], lhsT=ident[:],
                                         rhs=msg_t[:, cc, :],
                                         start=(d == 0), stop=(d == lt - 1))
                    # ---- dense epilogue for tile t ----
                    tsl = slice(t * P, (t + 1) * P)
                    y_sb = ep.tile([P, CH], f32, tag="ysb")
                    nc.vector.tensor_copy(y_sb[:], psum_y[:])
                    psum_yT = ps.tile([CH, P], f32, tag="pp", space="PSUM")
                    nc.tensor.transpose(out=psum_yT[:], in_=y_sb[:],
                                        identity=ident[:])
                    yT_sb = ep.tile([CH, P], f32, tag="yTsb")
                    nc.vector.tensor_copy(yT_sb[:], psum_yT[:])
                    xcur_t = ep.tile([CH, P], f32, tag="xcur")
                    nc.sync.dma_start(out=xcur_t[:],
                                      in_=xt_dram[layer % 2][:, tsl])
                    psum_x = ps.tile([CH, P], f32, tag="pp", space="PSUM")
                    nc.tensor.matmul(psum_x[:], lhsT=wmats[:, 2 * layer, :],
                                     rhs=yT_sb[:], start=True, stop=False)
                    nc.tensor.matmul(psum_x[:], lhsT=wmats[:, 2 * layer + 1, :],
                                     rhs=xcur_t[:], start=False, stop=True)
                    xnT_t = ep.tile([CH, P], f32, tag="xnT")
                    nc.scalar.activation(
                        out=xnT_t[:], in_=psum_x[:],
                        func=mybir.ActivationFunctionType.Relu,
                        bias=bvecs[:, layer:layer + 1], scale=1.0)
                    if layer < 2:
                        nc.sync.dma_start(out=xt_dram[(layer + 1) % 2][:, tsl],
                                          in_=xnT_t[:])
                    psum_xr = ps.tile([P, CH], f32, tag="pp", space="PSUM")
                    nc.tensor.transpose(out=psum_xr[:], in_=xnT_t[:],
                                        identity=ident[:CH, :CH])
                    xr_sb = ep.tile([P, CH], f32, tag="xrsb")
                    nc.vector.tensor_copy(xr_sb[:], psum_xr[:])
                    if layer < 2:
                        nc.sync.dma_start(out=ag_in[layer][tsl, :], in_=xr_sb[:])
                    else:
                        # pooling: P-matrix built on the fly from gcol
                        gc_t = ep.tile([P, 1], f32, tag="gc")
                        nc.sync.dma_start(out=gc_t[:], in_=gcol_ext[t])
                        pm = ep.tile([P, GW], f32, tag="pm")
                        nc.vector.tensor_scalar(
                            out=pm[:], in0=iota_f[:], scalar1=gc_t[:],
                            scalar2=None, op0=mybir.AluOpType.is_equal)
                        nc.tensor.matmul(poolA[:], lhsT=xr_sb[:],
                                         rhs=pm[:, :512],
                                         start=(t == 0), stop=(t == TILES - 1))
                        nc.tensor.matmul(poolB[:], lhsT=xr_sb[:],
                                         rhs=pm[:, 512:],
                                         start=(t == 0), stop=(t == TILES - 1))
                if layer < 2:
                    nc.gpsimd.collective_compute(
                        "AllGather", mybir.AluOpType.bypass,
                        replica_groups=[list(range(NC))],
                        ins=[ag_in[layer][:]], outs=[ag_out[layer][:]])
                    nc.sync.dma_start(out=tables[layer + 1][:],
                                      in_=ag_out[layer][:])

            # ---- write partial pooling sums [GW, CH] ----
            for s in range(GW // P):
                src = poolA if s < 4 else poolB
                off = s * P if s < 4 else (s - 4) * P
                pt_sb = ep.tile([CH, P], f32, tag="ptsb")
                nc.vector.tensor_copy(pt_sb[:], src[:, off:off + P])
                psum_pr = ps.tile([P, CH], f32, tag="pp", space="PSUM")
                nc.tensor.transpose(out=psum_pr[:], in_=pt_sb[:],
                                    identity=ident[:CH, :CH])
                pr_sb = ep.tile([P, CH], f32, tag="prsb")
                nc.vector.tensor_copy(pr_sb[:], psum_pr[:])
                nc.sync.dma_start(out=part_in[s * P:(s + 1) * P, :],
                                  in_=pr_sb[:])
            nc.gpsimd.collective_compute(
                "AllGather", mybir.AluOpType.bypass,
                replica_groups=[list(range(NC))],
                ins=[part_in[:]], outs=[part_out[:]])
            nc.sync.dma_start(
                out=part_int[:], in_=part_out[:].rearrange("(c g) d -> c g d", c=NC))

            # ---- combine partials, apply head, write all 4096 outputs ----
            for k in range(G // P):
                acc = ep.tile([P, CH], f32, tag="acc")
                for i, (c2, off) in enumerate(contrib[k]):
                    pt = ep.tile([P, CH], f32, tag="pt")
                    nc.sync.dma_start(out=pt[:],
                                      in_=part_int[c2, off:off + P, :])
                    if i == 0:
                        nc.vector.tensor_copy(acc[:], pt[:])
                    else:
                        nc.vector.tensor_add(acc[:], acc[:], pt[:])
                psum_aT = ps.tile([CH, P], f32, tag="pp", space="PSUM")
                nc.tensor.transpose(out=psum_aT[:], in_=acc[:],
                                    identity=ident[:])
                aT_sb = ep.tile([CH, P], f32, tag="aTsb")
                nc.vector.tensor_copy(aT_sb[:], psum_aT[:])
                psum_o = ps.tile([P, 1], f32, tag="pp", space="PSUM")
                nc.tensor.matmul(psum_o[:], lhsT=aT_sb[:], rhs=wd_t[:],
                                 start=True, stop=True)
                inv_t = ep.tile([P, 1], f32, tag="inv")
                nc.sync.dma_start(out=inv_t[:],
                                  in_=invcnt_ext[k * P:(k + 1) * P, :])
                logit = ep.tile([P, 1], f32, tag="logit")
                nc.vector.tensor_scalar(
                    out=logit[:], in0=psum_o[:], scalar1=inv_t[:],
                    scalar2=bd_t[:], op0=mybir.AluOpType.mult,
                    op1=mybir.AluOpType.add)
                o_sb = ep.tile([P, 1], f32, tag="osb")
                nc.scalar.activation(
                    out=o_sb[:], in_=logit[:],
                    func=mybir.ActivationFunctionType.Sigmoid,
                    bias=0.0, scale=1.0)
                nc.sync.dma_start(out=out_ext[k * P:(k + 1) * P, :], in_=o_sb[:])

    nc.finalize()
    return nc


def kernel(x, edge_src, edge_dst, edge_w, seg,
           W1_1, W2_1, b1, W1_2, W2_2, b2, W1_3, W2_3, b3, Wd, bd):
    global LAST_EXEC_NS
    in_maps_c, shared, meta = _preprocess(x, edge_src, edge_dst, edge_w, seg)

    W1_1p = np.zeros((CH, CH), np.float32)
    W1_1p[0] = np.asarray(W1_1, np.float32)[0]
    W2_1p = np.zeros((CH, CH), np.float32)
    W2_1p[0] = np.asarray(W2_1, np.float32)[0]
    wmat = np.ascontiguousarray(np.stack([
        W1_1p, W2_1p,
        np.asarray(W1_2, np.float32), np.asarray(W2_2, np.float32),
        np.asarray(W1_3, np.float32), np.asarray(W2_3, np.float32)
    ]).transpose(1, 0, 2))  # [CH(row), 6, CH(col)] — K on partitions
    bvec = np.ascontiguousarray(np.stack([
        np.asarray(b1, np.float32).reshape(CH),
        np.asarray(b2, np.float32).reshape(CH),
        np.asarray(b3, np.float32).reshape(CH)]).T)  # [CH, 3]
    wd = np.asarray(Wd, np.float32).reshape(CH, 1)
    bd_col = np.full((P, 1), float(np.asarray(bd).reshape(-1)[0]), np.float32)

    nc = _build(meta, None)
    in_maps = []
    for c in range(NC):
        m = dict(in_maps_c[c])
        m["x0p"] = shared["x0p"]
        m["invcnt"] = shared["invcnt"]
        m["wmat"] = wmat
        m["bvec"] = bvec
        m["wd"] = wd
        m["bd"] = bd_col
        in_maps.append(m)

    trace = TRACE and _install_ntff_hook()
    res = run_bass_kernel_spmd(nc, in_maps, core_ids=list(range(NC)),
                               trace=trace)
    LAST_EXEC_NS = res.exec_time_ns
    return np.asarray(res.results[0]["out"], np.float32)
